# revision 13
# baseline (speedup 1.0000x reference)
"""FF-sharded MoE FFN kernel for Trainium2 (8 NeuronCores), v2 "W8".

Strategy (pure FF-tensor-parallel, single group):
  - Host computes the gate in fp32 (softmax -> top-2 -> renormalize).
  - Every core processes ALL routed (expert, token) visits; the FFN
    hidden dim (FF=4096) is sharded 8 ways: core c holds columns
    [c*512, (c+1)*512) of every expert's W1 and the matching rows of
    W2, and computes
        Ypart = gelu(X @ W1[:, shard] + b1[shard]) @ W2[shard, :]
    for each expert segment. The host sums the 8 partials, applies the
    top-2 combine weights, and adds the b2 term.
  - Why: per-core work is exactly sum(counts)/8 * H * FFS MAC columns
    for ANY routing - zero load imbalance and zero slot padding (the
    previous expert-pairing scheme padded ~1%). HBM traffic is
    ~50MB/core (16 W + 17 x + 17 y), well under the ~95us of DMA a
    ~265us all-matmul kernel can hide.

Per-core schedule (8 segments = experts, descending token count):
  Segment inputs live in ONE DRAM tensor packed PARTITION-MAJOR: for
  each SBUF partition p, each segment's [w1 shard | x^T] block is a
  single contiguous run ordered [k][col] (k = contraction chunk). A
  full-segment input DMA is therefore 128 descriptors of ~25KB - near
  peak HBM bandwidth (the previous [H, cols] layout produced 1024
  256B-4KB descriptors; 256B descriptors measured only ~48GB/s).
  Segment 0 is split into 4 separately-DMA'd tiles (w1 ff-chunk 0,
  x block 0, w1 ff-chunks 1-3, x rest) so the first GEMM can start
  ~3us earlier; zero-matmul warm-up rides the initial DMA wait and
  ramps the PE out of its cold HAM state.
  All GEMMs in bf16 on the PE with fp32 PSUM accumulation; gelu
  (exact) is fused into the GEMM1 PSUM eviction with the b1 bias;
  GEMM2 partial outputs are written as bf16 (host sums in fp32). The
  last block's output DMA is staggered per-128-row chunk so only a
  ~0.2MB transfer trails the final matmul.
"""

import sys

if "/opt/trn_rl_repo" not in sys.path:
    sys.path.insert(0, "/opt/trn_rl_repo")

import numpy as np
import ml_dtypes

H = 1024          # hidden size
E = 8             # experts
TOPK = 2
FF = 4 * H        # expert hidden dim
P = 128           # SBUF partitions
NC = 8            # cores == FF shards
FFS = FF // NC    # per-core FF shard (512)
KH = H // P       # 8  contraction chunks for GEMM1
KFS = FFS // P    # 4  contraction chunks for GEMM2 (shard)
NB0 = 512         # segment-0 first block

_prog_cache: dict[tuple, object] = {}
LAST_RESULTS = None  # BassKernelResults of the most recent run (for test harness)
TRACE = False        # test harness can set kernel.TRACE = True for profiling
ACT_OVERRIDE = None  # sim-only: CoreSim lacks Gelu; tests may set e.g. "Relu"
LAST_CALL = None     # (nc, in_maps) of the most recent run, for re-runs
WARM_N = 16          # HAM/pstate pre-warm zero-matmuls at kernel start


def _seg_blocks(A: int, first: int | None = None):
    """Split A token columns into near-equal blocks <= 512.

    first: size of the first block (segment 0 only; small so its DMA
    lands early). Avoid blocks < ~230: below that LDWEIGHTS (~114ns)
    stops hiding behind the matmul stream.
    """
    blocks = []
    t = 0
    if first is not None:
        first = min(first, A)
        blocks.append((0, first))
        t = first
        A -= first
    if A > 0:
        nblk = -(-A // 512)
        base = A // nblk
        rem = A % nblk
        for i in range(nblk):
            nb = base + (1 if i < rem else 0)
            blocks.append((t, nb))
            t += nb
    return blocks


def _build_program(segs: tuple[int, ...]):
    """Build + compile the per-core SPMD Bass program.

    segs: token count per segment, descending (exact per-expert counts;
    identical on all cores).

    DRAM I/O (S = len(segs), Ctot = sum(segs)):
      xw  [P, 8*(S*FFS + Ctot)] bf16  partition-major packed inputs:
          per partition, per segment: [k][w1 cols | x cols] contiguous
          (segment 0 reordered into its 4 head chunks, see below)
      w2  [P, S*KFS*H] bf16  partition-major W2 shards: per partition,
          per segment: [k][h] contiguous
      b1p [P, S*KFS]  f32   b1 shard, col si*KFS+f = b1[f*128:(f+1)*128]
      y   [H, Ctot]   bf16  partial YT (unscaled, host sums all cores)
    """
    from contextlib import ExitStack

    from concourse import bacc
    import concourse.mybir as mybir
    import concourse.tile as tile

    dt = mybir.dt
    S = len(segs)
    Ctot = sum(segs)
    A0 = segs[0]
    nb0 = min(NB0, A0)

    def blocks_for(si):
        if si == 0:
            return _seg_blocks(segs[si], first=nb0)
        return _seg_blocks(segs[si])

    NBMAX = max(nb for si in range(S) for _, nb in blocks_for(si))

    nc = bacc.Bacc(None, target_bir_lowering=False, debug=False)

    xw = nc.dram_tensor("xw", [P, KH * (S * FFS + Ctot)], dt.bfloat16,
                        kind="ExternalInput")
    w2 = nc.dram_tensor("w2", [P, S * KFS * H], dt.bfloat16,
                        kind="ExternalInput")
    b1p = nc.dram_tensor("b1p", [P, S * KFS], dt.float32, kind="ExternalInput")
    y = nc.dram_tensor("y", [H, Ctot], dt.bfloat16, kind="ExternalOutput")

    y_r = y[:, :].rearrange("(k p) t -> p k t", p=P)

    # xw element offset (per partition) of each segment's packed block
    seg_off = [0]
    out_off = [0]
    for A in segs:
        seg_off.append(seg_off[-1] + KH * (FFS + A))
        out_off.append(out_off[-1] + A)

    def xw_src(seg_elem_off: int, ncols: int):
        """[p, k, c] view of a contiguous per-partition run of xw."""
        a = seg_elem_off
        return xw[:, a:a + KH * ncols].rearrange("p (k c) -> p k c", k=KH)

    with ExitStack() as ctx:
        tc = ctx.enter_context(tile.TileContext(nc))
        xwpool = ctx.enter_context(tc.tile_pool(name="xwpool", bufs=2))
        w2pool = ctx.enter_context(tc.tile_pool(name="w2pool", bufs=2))
        bpool = ctx.enter_context(tc.tile_pool(name="bpool", bufs=1))
        hpool = ctx.enter_context(tc.tile_pool(name="hpool", bufs=2))
        psA = ctx.enter_context(tc.tile_pool(name="psA", bufs=4, space="PSUM"))
        psB = ctx.enter_context(tc.tile_pool(name="psB", bufs=4, space="PSUM"))
        opool = ctx.enter_context(tc.tile_pool(name="opool", bufs=3))

        act = getattr(mybir.ActivationFunctionType, ACT_OVERRIDE or "Gelu")
        tiles = {}

        b1t = bpool.tile([P, S * KFS], dt.float32, tag="b1t", name="b1t")

        # --- segment 0: 4 one-shot tiles, DMA'd in consumption order ---
        # xw layout for seg 0 (per partition, element offsets from 0):
        #   ctW  [k][128]      w1 ff-chunk 0
        #   ctX  [k][nb0]      x block 0
        #   ctB1 [k][FFS-128]  w1 ff-chunks 1..KFS-1
        #   ctB2 [k][A0-nb0]   x rest
        ctW = bpool.tile([P, KH, P], dt.bfloat16, tag="ctW", name="ctW")
        ctX = bpool.tile([P, KH, nb0], dt.bfloat16, tag="ctX", name="ctX")
        ctB1a = bpool.tile([P, KH, P], dt.bfloat16, tag="ctB1a", name="ctB1a")
        ctB1b = bpool.tile([P, KH, FFS - 2 * P], dt.bfloat16, tag="ctB1b",
                           name="ctB1b")
        ctB2 = None
        if A0 > nb0:
            ctB2 = bpool.tile([P, KH, A0 - nb0], dt.bfloat16, tag="ctB2",
                              name="ctB2")

        def emit_seg0():
            # Head inputs spread across the 3 DMA-capable queues (sync,
            # scalar: HW-DGE ~130-165GB/s; gpsimd: SW-DGE ~84GB/s) in
            # consumption order so each piece lands just before its first
            # matmul needs it.
            o = 0
            nc.scalar.dma_start(out=ctW[:, :, :], in_=xw_src(o, P))
            o += KH * P
            nc.sync.dma_start(out=ctX[:, :, :], in_=xw_src(o, nb0))
            o += KH * nb0
            nc.gpsimd.dma_start(out=b1t[:], in_=b1p[:, :])
            nc.scalar.dma_start(out=ctB1a[:, :, :], in_=xw_src(o, P))
            o += KH * P
            nc.scalar.dma_start(out=ctB1b[:, :, :], in_=xw_src(o, FFS - 2 * P))
            o += KH * (FFS - 2 * P)
            if ctB2 is not None:
                nc.sync.dma_start(out=ctB2[:, :, :], in_=xw_src(o, A0 - nb0))
            emit_w2(0)

        def emit_w2(si):
            w2t = w2pool.tile([P, KFS, H], dt.bfloat16, tag="w2t",
                              name=f"w2t{si}")
            tiles[("w2", si)] = w2t
            src = w2[:, si * KFS * H:(si + 1) * KFS * H].rearrange(
                "p (k h) -> p k h", k=KFS)
            nc.gpsimd.dma_start(out=w2t[:, :, :], in_=src)

        def emit_seg(si):
            A = segs[si]
            ct = xwpool.tile([P, KH, FFS + A], dt.bfloat16, tag="ct",
                             name=f"ct{si}")
            tiles[("ct", si)] = ct
            nc.sync.dma_start(out=ct[:, :, :], in_=xw_src(seg_off[si], FFS + A))
            emit_w2(si)

        def lhsT1(si, k, ff):
            """GEMM1 stationary operand: w1 ff-chunk (128 cols)."""
            if si == 0:
                if ff == 0:
                    return ctW[:, k, :]
                if ff == 1:
                    return ctB1a[:, k, :]
                return ctB1b[:, k, (ff - 2) * P:(ff - 1) * P]
            ct = tiles[("ct", si)]
            return ct[:, k, ff * P:(ff + 1) * P]

        def rhs1(si, k, t0, nb):
            """GEMM1 moving operand: x token block."""
            if si == 0:
                if t0 < nb0:
                    return ctX[:, k, t0:t0 + nb]
                return ctB2[:, k, t0 - nb0:t0 - nb0 + nb]
            ct = tiles[("ct", si)]
            return ct[:, k, FFS + t0:FFS + t0 + nb]

        # warm-up zero tile first in the vector queue (no input deps) so
        # the PE can start ramping before any DMA lands
        warm = bpool.tile([P, NBMAX], dt.bfloat16, tag="warm", name="warm")
        nc.vector.memset(warm[:, :], 0.0)

        emit_seg0()
        if S > 1:
            emit_seg(1)

        for si, A in enumerate(segs):
            o0 = out_off[si]
            blocks = blocks_for(si)

            # --- GEMM1: HmidT[f, t] = gelu(sum_k W1[h,f]*xt[h,t] + b1[f])
            hblk = hpool.tile([P, KFS, A], dt.bfloat16, tag="hblk",
                              name=f"hblk{si}")
            for bi, (t0, nb) in enumerate(blocks):
                for ff in range(KFS):
                    pa = psA.tile([P, NBMAX], dt.float32, tag="pa",
                                  name=f"pa{si}_{bi}_{ff}")
                    warm_n = 0
                    if si == 0 and bi == 0 and ff == 0:
                        # Pre-warm: accumulate zero-matmuls into the first
                        # PSUM group while the first input DMAs land; also
                        # ramps the PE clock out of its cold p-state.
                        warm_n = WARM_N
                        for i in range(warm_n):
                            nc.tensor.matmul(
                                pa[:, :nb],
                                lhsT=warm[:, :P],
                                rhs=warm[:, :nb],
                                start=(i == 0),
                                stop=False,
                            )
                    for k in range(KH):
                        nc.tensor.matmul(
                            pa[:, :nb],
                            lhsT=lhsT1(si, k, ff),
                            rhs=rhs1(si, k, t0, nb),
                            start=(k == 0 and warm_n == 0),
                            stop=(k == KH - 1),
                        )
                    nc.scalar.activation(
                        hblk[:, ff, t0:t0 + nb],
                        pa[:, :nb],
                        act,
                        bias=b1t[:, si * KFS + ff:si * KFS + ff + 1],
                    )
                if bi == 0 and si + 1 < S and si > 0:
                    # Prefetch segment si+1 while the rest of this segment
                    # computes (~28us of cover for ~4MB).
                    emit_seg(si + 1)

            # --- GEMM2: YT[h, t] = sum_f W2[f, h] * HmidT[f, t] -----------
            w2t = tiles.pop(("w2", si))
            for bi, (t0, nb) in enumerate(blocks):
                ot = opool.tile([P, KH, NBMAX], dt.bfloat16, tag="ot",
                                name=f"ot{si}_{bi}")
                # tail: the last two blocks drain in staggered chunks on
                # rotating queues so the transfers overlap the remaining
                # matmuls and each other (one queue moves only ~165GB/s)
                nblk_left = len(blocks) - bi if si == S - 1 else 99
                if nblk_left == 1:       # final block: 2-ht chunks
                    stagger = {1: (0, nc.scalar), 3: (2, nc.sync),
                               5: (4, nc.scalar), 7: (6, nc.sync)}
                elif nblk_left == 2:     # second-to-last: 4-ht chunks
                    stagger = {3: (0, nc.scalar), 7: (4, nc.sync)}
                else:
                    stagger = None
                for ht in range(KH):
                    pb = psB.tile([P, NBMAX], dt.float32, tag="pb",
                                  name=f"pb{si}_{bi}_{ht}")
                    for k in range(KFS):
                        nc.tensor.matmul(
                            pb[:, :nb],
                            lhsT=w2t[:, k, ht * P:(ht + 1) * P],
                            rhs=hblk[:, k, t0:t0 + nb],
                            start=(k == 0),
                            stop=(k == KFS - 1),
                        )
                    nc.vector.tensor_copy(ot[:, ht, :nb], pb[:, :nb])
                    if stagger is not None and ht in stagger:
                        lo, eng = stagger[ht]
                        eng.dma_start(
                            out=y_r[:, lo:ht + 1, o0 + t0:o0 + t0 + nb],
                            in_=ot[:, lo:ht + 1, :nb],
                        )
                if stagger is None:
                    nc.scalar.dma_start(
                        out=y_r[:, :, o0 + t0:o0 + t0 + nb],
                        in_=ot[:, :, :nb],
                    )

    nc.compile()
    return nc


def _get_program(segs: tuple[int, ...]):
    if segs not in _prog_cache:
        _prog_cache[segs] = _build_program(segs)
    return _prog_cache[segs]


def _route(xf: np.ndarray, Wg: np.ndarray, bg: np.ndarray):
    """fp32 gate: softmax -> top-2 (stable order, matches jax top_k) -> renorm."""
    logits = xf @ np.asarray(Wg, np.float32) + np.asarray(bg, np.float32)
    m = logits.max(axis=1, keepdims=True)
    p = np.exp(logits - m, dtype=np.float32)
    p /= p.sum(axis=1, keepdims=True)
    order = np.argsort(-p, axis=1, kind="stable")
    idx = order[:, :TOPK]
    pv = np.take_along_axis(p, idx, axis=1)
    vals = (pv / pv.sum(axis=1, keepdims=True)).astype(np.float32)
    return idx, vals


def _pack_pm(arr_hc: np.ndarray) -> np.ndarray:
    """[H, C] -> partition-major [P, KH*C] (per partition: [k][c])."""
    h, c = arr_hc.shape
    return np.ascontiguousarray(
        arr_hc.reshape(h // P, P, c).transpose(1, 0, 2).reshape(P, -1)
    )


def kernel(x, Wg, bg, W1, b1, W2, b2):
    global LAST_RESULTS, LAST_CALL
    from concourse.bass_utils import run_bass_kernel_spmd

    bf16 = ml_dtypes.bfloat16
    x = np.asarray(x, np.float32)
    xf = x.reshape(-1, H)
    T = xf.shape[0]

    idx, vals = _route(xf, Wg, bg)
    counts = np.bincount(idx.ravel(), minlength=E)

    # Segments: experts by token count (desc), zero-count experts skipped.
    order = [int(e) for e in np.argsort(-counts, kind="stable") if counts[e] > 0]
    segs = tuple(int(counts[e]) for e in order)
    S = len(segs)
    Ctot = sum(segs)
    A0 = segs[0]
    nb0 = min(NB0, A0)

    nc = _get_program(segs)

    W1 = np.asarray(W1, np.float32)
    W2 = np.asarray(W2, np.float32)
    b1 = np.asarray(b1, np.float32)

    # Token ids / combine scales / packed x^T per segment (shared by cores).
    shards = []
    xparts = []   # per segment: [P, KH*A] partition-major bf16
    for si in range(S):
        e = order[si]
        sel = idx == e                  # [T, 2]; at most one True per row
        ids = np.nonzero(sel.any(axis=1))[0]
        sc = vals[sel]                  # row-major => aligned with ids
        shards.append((ids, sc))
        xparts.append(_pack_pm(xf[ids].T.astype(bf16)))

    in_maps = []
    for c in range(NC):
        pieces = []
        for si in range(S):
            e = order[si]
            w1s = W1[e][:, c * FFS:(c + 1) * FFS].astype(bf16)
            if si == 0:
                # head chunk order (each piece k-major to match its SBUF
                # tile): w1 ff0 | x blk0 | w1 ff1 | w1 ff2.. | x rest
                x3 = xparts[0].reshape(P, KH, A0)
                pieces.append(_pack_pm(w1s[:, :P]))
                pieces.append(np.ascontiguousarray(x3[:, :, :nb0])
                              .reshape(P, -1))
                pieces.append(_pack_pm(w1s[:, P:2 * P]))
                pieces.append(_pack_pm(w1s[:, 2 * P:]))
                if A0 > nb0:
                    pieces.append(np.ascontiguousarray(x3[:, :, nb0:])
                                  .reshape(P, -1))
            else:
                # per partition: [k][w1 cols | x cols] contiguous
                w13 = _pack_pm(w1s).reshape(P, KH, FFS)
                x3 = xparts[si].reshape(P, KH, segs[si])
                pieces.append(np.concatenate([w13, x3], axis=2)
                              .reshape(P, -1))
        xwc = np.ascontiguousarray(np.concatenate(pieces, axis=1))
        w2c = np.concatenate(
            [_pack_pm(W2[order[si]][c * FFS:(c + 1) * FFS, :].astype(bf16))
             for si in range(S)],
            axis=1,
        )
        b1c = np.ascontiguousarray(np.stack(
            [b1[order[si]][c * FFS + f * P:c * FFS + (f + 1) * P]
             for si in range(S) for f in range(KFS)],
            axis=1,
        ))
        in_maps.append({"xw": xwc, "w2": np.ascontiguousarray(w2c), "b1p": b1c})

    LAST_CALL = (nc, in_maps)
    LAST_RESULTS = run_bass_kernel_spmd(nc, in_maps, list(range(NC)),
                                        trace=TRACE)

    ysum = np.zeros((H, Ctot), np.float32)
    for c in range(NC):
        ysum += LAST_RESULTS.results[c]["y"].astype(np.float32)

    out = np.zeros((T, H), np.float32)
    c0 = 0
    for si in range(S):
        ids, sc = shards[si]
        out[ids] += ysum[:, c0:c0 + ids.size].T * sc[:, None]
        c0 += segs[si]

    b2 = np.asarray(b2, np.float32)
    out += vals[:, 0:1] * b2[idx[:, 0]] + vals[:, 1:2] * b2[idx[:, 1]]
    return out.reshape(x.shape)


# revision 14
# speedup vs baseline: 1.0816x; 1.0816x over previous
"""FF-sharded MoE FFN kernel for Trainium2 (8 NeuronCores), v2 "W8".

Strategy (pure FF-tensor-parallel, single group):
  - Host computes the gate in fp32 (softmax -> top-2 -> renormalize).
  - Every core processes ALL routed (expert, token) visits; the FFN
    hidden dim (FF=4096) is sharded 8 ways: core c holds columns
    [c*512, (c+1)*512) of every expert's W1 and the matching rows of
    W2, and computes
        Ypart = gelu(X @ W1[:, shard] + b1[shard]) @ W2[shard, :]
    for each expert segment. The host sums the 8 partials, applies the
    top-2 combine weights, and adds the b2 term.
  - Why: per-core work is exactly sum(counts)/8 * H * FFS MAC columns
    for ANY routing - zero load imbalance and zero slot padding (the
    previous expert-pairing scheme padded ~1%). HBM traffic is
    ~50MB/core (16 W + 17 x + 17 y), hidden under ~265us of matmul.

Per-core schedule (8 segments = experts, descending token count):
  Inputs are packed PARTITION-MAJOR: per SBUF partition, each DMA'd
  piece is one contiguous [k][col] run, so every transfer is 128 large
  descriptors (small strided descriptors measured as low as 37GB/s;
  large ones ~245GB/s). Queue assignment is driven by measured queue
  rates: the sync queue is the only fast one (~245GB/s), so ALL
  latency-critical input (w1|x) and output (y) traffic goes to sync in
  exact consumption order; w2 and b1 (needed one phase later) ride the
  ~80GB/s gpsimd SW-DGE queue. Segment 0 is split into per-ff-chunk /
  per-block tiles so the first GEMMs can start as soon as ~1MB has
  landed; zero-matmul warm-up covers the initial DMA wait and ramps
  the PE out of its cold HAM state (cold start costs ~2x for ~3.4us).
  All GEMMs bf16 on the PE with fp32 PSUM accumulation; exact gelu is
  fused into the GEMM1 PSUM eviction (ScalarE) with the b1 bias; GEMM2
  evictions (VectorE) write bf16 into a per-block PACKED staging tile
  so the y output DMA is one contiguous run per partition. The last
  two blocks drain in staggered 2/4-row-chunk DMAs so only ~0.25MB of
  transfer trails the final matmul.
"""

import sys

if "/opt/trn_rl_repo" not in sys.path:
    sys.path.insert(0, "/opt/trn_rl_repo")

import numpy as np
import ml_dtypes

H = 1024          # hidden size
E = 8             # experts
TOPK = 2
FF = 4 * H        # expert hidden dim
P = 128           # SBUF partitions
NC = 8            # cores == FF shards
FFS = FF // NC    # per-core FF shard (512)
KH = H // P       # 8  contraction chunks for GEMM1
KFS = FFS // P    # 4  contraction chunks for GEMM2 (shard)
NB0 = 512         # segment-0 first block

_prog_cache: dict[tuple, object] = {}
LAST_RESULTS = None  # BassKernelResults of the most recent run (for test harness)
TRACE = False        # test harness can set kernel.TRACE = True for profiling
ACT_OVERRIDE = None  # sim-only: CoreSim lacks Gelu; tests may set e.g. "Relu"
LAST_CALL = None     # (nc, in_maps) of the most recent run, for re-runs
WARM_N = 20          # HAM/pstate pre-warm zero-matmuls at kernel start


def _seg_blocks(A: int, first: int | None = None):
    """Split A token columns into near-equal blocks <= 512.

    first: size of the first block (segment 0 only). Avoid blocks
    < ~230: below that LDWEIGHTS (~114ns) stops hiding behind the
    matmul stream.
    """
    blocks = []
    t = 0
    if first is not None:
        first = min(first, A)
        blocks.append((0, first))
        t = first
        A -= first
    if A > 0:
        nblk = -(-A // 512)
        base = A // nblk
        rem = A % nblk
        for i in range(nblk):
            nb = base + (1 if i < rem else 0)
            blocks.append((t, nb))
            t += nb
    return blocks


def _plan(segs: tuple[int, ...]):
    """Shared host/kernel plan: per-segment blocks + packed-y offsets.

    Returns (blocks_by_seg, boffs) where boffs[(si, bi)] is the element
    offset (per partition) of that block's [ht][t]-contiguous span in
    the packed y output.
    """
    nb0 = min(NB0, segs[0])
    blocks_by_seg = [
        _seg_blocks(A, first=nb0 if si == 0 else None)
        for si, A in enumerate(segs)
    ]
    boffs = {}
    off = 0
    for si, blocks in enumerate(blocks_by_seg):
        for bi, (t0, nb) in enumerate(blocks):
            boffs[(si, bi)] = off
            off += KH * nb
    return blocks_by_seg, boffs


def _build_program(segs: tuple[int, ...]):
    """Build + compile the per-core SPMD Bass program.

    segs: token count per segment, descending (exact per-expert counts;
    identical on all cores).

    DRAM I/O (S = len(segs), Ctot = sum(segs)):
      xw  [P, 8*(S*FFS + Ctot)] bf16  partition-major packed inputs:
          per partition, per piece: [k][cols] contiguous (segment 0 is
          stored as separate pieces: w1c0 | xblk0 | w1c1 | w1c2.. |
          xblk1 | xblk2 ..)
      w2  [P, S*KFS*H] bf16  partition-major W2 shards
      b1p [P, S*KFS]  f32   b1 shard, col si*KFS+f = b1[f*128:(f+1)*128]
      y   [P, KH*Ctot] bf16 partial YT, packed per block: each block's
          span is [ht][t] contiguous per partition (host sums cores,
          then unpacks)
    """
    from contextlib import ExitStack

    from concourse import bacc
    import concourse.mybir as mybir
    import concourse.tile as tile

    dt = mybir.dt
    S = len(segs)
    Ctot = sum(segs)
    A0 = segs[0]
    nb0 = min(NB0, A0)
    blocks_by_seg, boffs = _plan(segs)
    NBMAX = max(nb for blocks in blocks_by_seg for _, nb in blocks)

    nc = bacc.Bacc(None, target_bir_lowering=False, debug=False)

    xw = nc.dram_tensor("xw", [P, KH * (S * FFS + Ctot)], dt.bfloat16,
                        kind="ExternalInput")
    w2 = nc.dram_tensor("w2", [P, S * KFS * H], dt.bfloat16,
                        kind="ExternalInput")
    b1p = nc.dram_tensor("b1p", [P, S * KFS], dt.float32, kind="ExternalInput")
    y = nc.dram_tensor("y", [P, KH * Ctot], dt.bfloat16, kind="ExternalOutput")

    # xw element offset (per partition) of each segment's packed block;
    # segment 0 occupies [0, KH*(FFS+A0)) split into its pieces.
    seg_off = [0]
    for A in segs:
        seg_off.append(seg_off[-1] + KH * (FFS + A))

    def xw_src(elem_off: int, ncols: int):
        """[p, k, c] view of a contiguous per-partition run of xw."""
        a = elem_off
        return xw[:, a:a + KH * ncols].rearrange("p (k c) -> p k c", k=KH)

    with ExitStack() as ctx:
        tc = ctx.enter_context(tile.TileContext(nc))
        xwpool = ctx.enter_context(tc.tile_pool(name="xwpool", bufs=2))
        w2pool = ctx.enter_context(tc.tile_pool(name="w2pool", bufs=2))
        bpool = ctx.enter_context(tc.tile_pool(name="bpool", bufs=1))
        hpool = ctx.enter_context(tc.tile_pool(name="hpool", bufs=2))
        psA = ctx.enter_context(tc.tile_pool(name="psA", bufs=4, space="PSUM"))
        psB = ctx.enter_context(tc.tile_pool(name="psB", bufs=4, space="PSUM"))
        opool = ctx.enter_context(tc.tile_pool(name="opool", bufs=3))

        act = getattr(mybir.ActivationFunctionType, ACT_OVERRIDE or "Gelu")
        tiles = {}

        b1t = bpool.tile([P, S * KFS], dt.float32, tag="b1t", name="b1t")

        # --- segment 0: one tile per piece, all on the fast sync queue
        # in exact consumption order ---
        blocks0 = blocks_by_seg[0]
        ctW = bpool.tile([P, KH, P], dt.bfloat16, tag="ctW", name="ctW")
        ctB1a = bpool.tile([P, KH, P], dt.bfloat16, tag="ctB1a", name="ctB1a")
        ctB1b = bpool.tile([P, KH, FFS - 2 * P], dt.bfloat16, tag="ctB1b",
                           name="ctB1b")
        xts = [bpool.tile([P, KH, nb], dt.bfloat16, tag=f"xt{bi}",
                          name=f"xt{bi}")
               for bi, (t0, nb) in enumerate(blocks0)]

        def emit_seg0():
            o = 0
            nc.sync.dma_start(out=ctW[:, :, :], in_=xw_src(o, P))
            o += KH * P
            nc.sync.dma_start(out=xts[0][:, :, :], in_=xw_src(o, blocks0[0][1]))
            o += KH * blocks0[0][1]
            nc.gpsimd.dma_start(out=b1t[:], in_=b1p[:, :])
            nc.sync.dma_start(out=ctB1a[:, :, :], in_=xw_src(o, P))
            o += KH * P
            nc.sync.dma_start(out=ctB1b[:, :, :], in_=xw_src(o, FFS - 2 * P))
            o += KH * (FFS - 2 * P)
            for bi in range(1, len(blocks0)):
                nb = blocks0[bi][1]
                nc.sync.dma_start(out=xts[bi][:, :, :], in_=xw_src(o, nb))
                o += KH * nb
            emit_w2(0)

        def emit_w2(si):
            w2t = w2pool.tile([P, KFS, H], dt.bfloat16, tag="w2t",
                              name=f"w2t{si}")
            tiles[("w2", si)] = w2t
            src = w2[:, si * KFS * H:(si + 1) * KFS * H].rearrange(
                "p (k h) -> p k h", k=KFS)
            nc.gpsimd.dma_start(out=w2t[:, :, :], in_=src)

        def emit_seg(si):
            A = segs[si]
            ct = xwpool.tile([P, KH, FFS + A], dt.bfloat16, tag="ct",
                             name=f"ct{si}")
            tiles[("ct", si)] = ct
            nc.sync.dma_start(out=ct[:, :, :], in_=xw_src(seg_off[si], FFS + A))
            emit_w2(si)

        def lhsT1(si, k, ff):
            """GEMM1 stationary operand: w1 ff-chunk (128 cols)."""
            if si == 0:
                if ff == 0:
                    return ctW[:, k, :]
                if ff == 1:
                    return ctB1a[:, k, :]
                return ctB1b[:, k, (ff - 2) * P:(ff - 1) * P]
            ct = tiles[("ct", si)]
            return ct[:, k, ff * P:(ff + 1) * P]

        def rhs1(si, k, bi, t0, nb):
            """GEMM1 moving operand: x token block."""
            if si == 0:
                return xts[bi][:, k, :nb]
            ct = tiles[("ct", si)]
            return ct[:, k, FFS + t0:FFS + t0 + nb]

        # warm-up zero tile first in the vector queue (no input deps) so
        # the PE can start ramping before any DMA lands
        warm = bpool.tile([P, NBMAX], dt.bfloat16, tag="warm", name="warm")
        nc.vector.memset(warm[:, :], 0.0)

        emit_seg0()
        if S > 1:
            emit_seg(1)

        for si, A in enumerate(segs):
            blocks = blocks_by_seg[si]

            # --- GEMM1: HmidT[f, t] = gelu(sum_k W1[h,f]*xt[h,t] + b1[f])
            hblk = hpool.tile([P, KFS, A], dt.bfloat16, tag="hblk",
                              name=f"hblk{si}")
            for bi, (t0, nb) in enumerate(blocks):
                for ff in range(KFS):
                    pa = psA.tile([P, NBMAX], dt.float32, tag="pa",
                                  name=f"pa{si}_{bi}_{ff}")
                    warm_n = 0
                    if si == 0 and bi == 0 and ff == 0:
                        # Pre-warm: accumulate zero-matmuls into the first
                        # PSUM group while the first input DMAs land; also
                        # ramps the PE clock out of its cold p-state.
                        warm_n = WARM_N
                        for i in range(warm_n):
                            nc.tensor.matmul(
                                pa[:, :nb],
                                lhsT=warm[:, :P],
                                rhs=warm[:, :nb],
                                start=(i == 0),
                                stop=False,
                            )
                    for k in range(KH):
                        nc.tensor.matmul(
                            pa[:, :nb],
                            lhsT=lhsT1(si, k, ff),
                            rhs=rhs1(si, k, bi, t0, nb),
                            start=(k == 0 and warm_n == 0),
                            stop=(k == KH - 1),
                        )
                    nc.scalar.activation(
                        hblk[:, ff, t0:t0 + nb],
                        pa[:, :nb],
                        act,
                        bias=b1t[:, si * KFS + ff:si * KFS + ff + 1],
                    )
                if bi == 0 and si + 1 < S and si > 0:
                    # Prefetch segment si+1 while the rest of this segment
                    # computes (~28us of cover for ~4MB).
                    emit_seg(si + 1)

            # --- GEMM2: YT[h, t] = sum_f W2[f, h] * HmidT[f, t] -----------
            w2t = tiles.pop(("w2", si))
            for bi, (t0, nb) in enumerate(blocks):
                boff = boffs[(si, bi)]
                # ot is PACKED at stride nb so the output DMA is one
                # contiguous [ht][t] run per partition (large descriptors)
                ot = opool.tile([P, KH * NBMAX], dt.bfloat16, tag="ot",
                                name=f"ot{si}_{bi}")
                # tail: the last two blocks drain in staggered row-chunks
                # so only a small transfer trails the final matmul
                nblk_left = len(blocks) - bi if si == S - 1 else 99
                if nblk_left == 1:       # final block: 2-ht chunks
                    stagger = {1: 0, 3: 2, 5: 4, 7: 6}
                elif nblk_left == 2:     # second-to-last: 4-ht chunks
                    stagger = {3: 0, 7: 4}
                else:
                    stagger = None
                for ht in range(KH):
                    pb = psB.tile([P, NBMAX], dt.float32, tag="pb",
                                  name=f"pb{si}_{bi}_{ht}")
                    for k in range(KFS):
                        nc.tensor.matmul(
                            pb[:, :nb],
                            lhsT=w2t[:, k, ht * P:(ht + 1) * P],
                            rhs=hblk[:, k, t0:t0 + nb],
                            start=(k == 0),
                            stop=(k == KFS - 1),
                        )
                    nc.vector.tensor_copy(ot[:, ht * nb:(ht + 1) * nb],
                                          pb[:, :nb])
                    if stagger is not None and ht in stagger:
                        lo = stagger[ht]
                        nc.sync.dma_start(
                            out=y[:, boff + lo * nb:boff + (ht + 1) * nb],
                            in_=ot[:, lo * nb:(ht + 1) * nb],
                        )
                if stagger is None:
                    nc.sync.dma_start(
                        out=y[:, boff:boff + KH * nb],
                        in_=ot[:, :KH * nb],
                    )

    nc.compile()
    return nc


def _get_program(segs: tuple[int, ...]):
    if segs not in _prog_cache:
        _prog_cache[segs] = _build_program(segs)
    return _prog_cache[segs]


def _route(xf: np.ndarray, Wg: np.ndarray, bg: np.ndarray):
    """fp32 gate: softmax -> top-2 (stable order, matches jax top_k) -> renorm."""
    logits = xf @ np.asarray(Wg, np.float32) + np.asarray(bg, np.float32)
    m = logits.max(axis=1, keepdims=True)
    p = np.exp(logits - m, dtype=np.float32)
    p /= p.sum(axis=1, keepdims=True)
    order = np.argsort(-p, axis=1, kind="stable")
    idx = order[:, :TOPK]
    pv = np.take_along_axis(p, idx, axis=1)
    vals = (pv / pv.sum(axis=1, keepdims=True)).astype(np.float32)
    return idx, vals


def _pack_pm(arr_hc: np.ndarray) -> np.ndarray:
    """[H, C] -> partition-major [P, KH*C] (per partition: [k][c])."""
    h, c = arr_hc.shape
    return np.ascontiguousarray(
        arr_hc.reshape(h // P, P, c).transpose(1, 0, 2).reshape(P, -1)
    )


def kernel(x, Wg, bg, W1, b1, W2, b2):
    global LAST_RESULTS, LAST_CALL
    from concourse.bass_utils import run_bass_kernel_spmd

    bf16 = ml_dtypes.bfloat16
    x = np.asarray(x, np.float32)
    xf = x.reshape(-1, H)
    T = xf.shape[0]

    idx, vals = _route(xf, Wg, bg)
    counts = np.bincount(idx.ravel(), minlength=E)

    # Segments: experts by token count (desc), zero-count experts skipped.
    order = [int(e) for e in np.argsort(-counts, kind="stable") if counts[e] > 0]
    segs = tuple(int(counts[e]) for e in order)
    S = len(segs)
    Ctot = sum(segs)
    A0 = segs[0]
    nb0 = min(NB0, A0)
    blocks_by_seg, boffs = _plan(segs)

    nc = _get_program(segs)

    W1 = np.asarray(W1, np.float32)
    W2 = np.asarray(W2, np.float32)
    b1 = np.asarray(b1, np.float32)

    # Token ids / combine scales / packed x^T per segment (shared by cores).
    shards = []
    xparts = []   # per segment: [P, KH*A] partition-major bf16
    for si in range(S):
        e = order[si]
        sel = idx == e                  # [T, 2]; at most one True per row
        ids = np.nonzero(sel.any(axis=1))[0]
        sc = vals[sel]                  # row-major => aligned with ids
        shards.append((ids, sc))
        xparts.append(_pack_pm(xf[ids].T.astype(bf16)))

    in_maps = []
    for c in range(NC):
        pieces = []
        for si in range(S):
            e = order[si]
            w1s = W1[e][:, c * FFS:(c + 1) * FFS].astype(bf16)
            if si == 0:
                # piece order (each k-major, matching its SBUF tile):
                # w1c0 | xblk0 | w1c1 | w1c2.. | xblk1 | xblk2 ..
                x3 = xparts[0].reshape(P, KH, A0)
                pieces.append(_pack_pm(w1s[:, :P]))
                pieces.append(np.ascontiguousarray(x3[:, :, :nb0])
                              .reshape(P, -1))
                pieces.append(_pack_pm(w1s[:, P:2 * P]))
                pieces.append(_pack_pm(w1s[:, 2 * P:]))
                for t0, nb in blocks_by_seg[0][1:]:
                    pieces.append(np.ascontiguousarray(x3[:, :, t0:t0 + nb])
                                  .reshape(P, -1))
            else:
                # per partition: [k][w1 cols | x cols] contiguous
                w13 = _pack_pm(w1s).reshape(P, KH, FFS)
                x3 = xparts[si].reshape(P, KH, segs[si])
                pieces.append(np.concatenate([w13, x3], axis=2)
                              .reshape(P, -1))
        xwc = np.ascontiguousarray(np.concatenate(pieces, axis=1))
        w2c = np.concatenate(
            [_pack_pm(W2[order[si]][c * FFS:(c + 1) * FFS, :].astype(bf16))
             for si in range(S)],
            axis=1,
        )
        b1c = np.ascontiguousarray(np.stack(
            [b1[order[si]][c * FFS + f * P:c * FFS + (f + 1) * P]
             for si in range(S) for f in range(KFS)],
            axis=1,
        ))
        in_maps.append({"xw": xwc, "w2": np.ascontiguousarray(w2c), "b1p": b1c})

    LAST_CALL = (nc, in_maps)
    LAST_RESULTS = run_bass_kernel_spmd(nc, in_maps, list(range(NC)),
                                        trace=TRACE)

    # Sum partials across cores in the packed layout, then unpack.
    ysum2 = np.zeros((P, KH * Ctot), np.float32)
    for c in range(NC):
        ysum2 += LAST_RESULTS.results[c]["y"].astype(np.float32)
    ysum = np.empty((H, Ctot), np.float32)
    soff = 0
    for si in range(S):
        for bi, (t0, nb) in enumerate(blocks_by_seg[si]):
            boff = boffs[(si, bi)]
            blk = ysum2[:, boff:boff + KH * nb].reshape(P, KH, nb)
            ysum[:, soff + t0:soff + t0 + nb] = (
                blk.transpose(1, 0, 2).reshape(H, nb))
        soff += segs[si]

    out = np.zeros((T, H), np.float32)
    c0 = 0
    for si in range(S):
        ids, sc = shards[si]
        out[ids] += ysum[:, c0:c0 + ids.size].T * sc[:, None]
        c0 += segs[si]

    b2 = np.asarray(b2, np.float32)
    out += vals[:, 0:1] * b2[idx[:, 0]] + vals[:, 1:2] * b2[idx[:, 1]]
    return out.reshape(x.shape)


# revision 15
# speedup vs baseline: 1.0832x; 1.0015x over previous
"""FF-sharded MoE FFN kernel for Trainium2 (8 NeuronCores), v2 "W8".

Strategy (pure FF-tensor-parallel, single group):
  - Host computes the gate in fp32 (softmax -> top-2 -> renormalize).
  - Every core processes ALL routed (expert, token) visits; the FFN
    hidden dim (FF=4096) is sharded 8 ways: core c holds columns
    [c*512, (c+1)*512) of every expert's W1 and the matching rows of
    W2, and computes
        Ypart = gelu(X @ W1[:, shard] + b1[shard]) @ W2[shard, :]
    for each expert segment. The host sums the 8 partials, applies the
    top-2 combine weights, and adds the b2 term.
  - Why: per-core work is exactly sum(counts)/8 * H * FFS MAC columns
    for ANY routing - zero load imbalance and zero slot padding (the
    previous expert-pairing scheme padded ~1%). HBM traffic is
    ~50MB/core (16 W + 17 x + 17 y), hidden under ~265us of matmul.

Per-core schedule (8 segments = experts, descending token count):
  Inputs are packed PARTITION-MAJOR: per SBUF partition, each DMA'd
  piece is one contiguous [k][col] run, so every transfer is 128 large
  descriptors (small strided descriptors measured as low as 37GB/s;
  large ones ~245GB/s). Queue assignment is driven by measured queue
  rates: the sync queue is the only fast one (~245GB/s), so ALL
  latency-critical input (w1|x) and output (y) traffic goes to sync in
  exact consumption order; w2 and b1 (needed one phase later) ride the
  ~80GB/s gpsimd SW-DGE queue. Segment 0 is split into per-ff-chunk /
  per-block tiles so the first GEMMs can start as soon as ~1MB has
  landed; zero-matmul warm-up covers the initial DMA wait and ramps
  the PE out of its cold HAM state (cold start costs ~2x for ~3.4us).
  All GEMMs bf16 on the PE with fp32 PSUM accumulation; exact gelu is
  fused into the GEMM1 PSUM eviction (ScalarE) with the b1 bias; GEMM2
  evictions (VectorE) write bf16 into a per-block PACKED staging tile
  so the y output DMA is one contiguous run per partition. The last
  two blocks drain in staggered 2/4-row-chunk DMAs so only ~0.25MB of
  transfer trails the final matmul.
"""

import sys

if "/opt/trn_rl_repo" not in sys.path:
    sys.path.insert(0, "/opt/trn_rl_repo")

import numpy as np
import ml_dtypes

H = 1024          # hidden size
E = 8             # experts
TOPK = 2
FF = 4 * H        # expert hidden dim
P = 128           # SBUF partitions
NC = 8            # cores == FF shards
FFS = FF // NC    # per-core FF shard (512)
KH = H // P       # 8  contraction chunks for GEMM1
KFS = FFS // P    # 4  contraction chunks for GEMM2 (shard)
NB0 = 512         # segment-0 first block

_prog_cache: dict[tuple, object] = {}
LAST_RESULTS = None  # BassKernelResults of the most recent run (for test harness)
TRACE = False        # test harness can set kernel.TRACE = True for profiling
ACT_OVERRIDE = None  # sim-only: CoreSim lacks Gelu; tests may set e.g. "Relu"
LAST_CALL = None     # (nc, in_maps) of the most recent run, for re-runs
WARM_N = 22          # HAM/pstate pre-warm zero-matmuls at kernel start


def _seg_blocks(A: int, first: int | None = None):
    """Split A token columns into near-equal blocks <= 512.

    first: size of the first block (segment 0 only). Avoid blocks
    < ~230: below that LDWEIGHTS (~114ns) stops hiding behind the
    matmul stream.
    """
    blocks = []
    t = 0
    if first is not None:
        first = min(first, A)
        blocks.append((0, first))
        t = first
        A -= first
    if A > 0:
        nblk = -(-A // 512)
        base = A // nblk
        rem = A % nblk
        for i in range(nblk):
            nb = base + (1 if i < rem else 0)
            blocks.append((t, nb))
            t += nb
    return blocks


def _plan(segs: tuple[int, ...]):
    """Shared host/kernel plan: per-segment blocks + packed-y offsets.

    Returns (blocks_by_seg, boffs) where boffs[(si, bi)] is the element
    offset (per partition) of that block's [ht][t]-contiguous span in
    the packed y output.
    """
    nb0 = min(NB0, segs[0])
    blocks_by_seg = [
        _seg_blocks(A, first=nb0 if si == 0 else None)
        for si, A in enumerate(segs)
    ]
    boffs = {}
    off = 0
    for si, blocks in enumerate(blocks_by_seg):
        for bi, (t0, nb) in enumerate(blocks):
            boffs[(si, bi)] = off
            off += KH * nb
    return blocks_by_seg, boffs


def _build_program(segs: tuple[int, ...]):
    """Build + compile the per-core SPMD Bass program.

    segs: token count per segment, descending (exact per-expert counts;
    identical on all cores).

    DRAM I/O (S = len(segs), Ctot = sum(segs)):
      xw  [P, 8*(S*FFS + Ctot)] bf16  partition-major packed inputs:
          per partition, per piece: [k][cols] contiguous (segment 0 is
          stored as separate pieces: w1c0 | xblk0 | w1c1 | w1c2.. |
          xblk1 | xblk2 ..)
      w2  [P, S*KFS*H] bf16  partition-major W2 shards
      b1p [P, S*KFS]  f32   b1 shard, col si*KFS+f = b1[f*128:(f+1)*128]
      y   [P, KH*Ctot] bf16 partial YT, packed per block: each block's
          span is [ht][t] contiguous per partition (host sums cores,
          then unpacks)
    """
    from contextlib import ExitStack

    from concourse import bacc
    import concourse.mybir as mybir
    import concourse.tile as tile

    dt = mybir.dt
    S = len(segs)
    Ctot = sum(segs)
    A0 = segs[0]
    nb0 = min(NB0, A0)
    blocks_by_seg, boffs = _plan(segs)
    NBMAX = max(nb for blocks in blocks_by_seg for _, nb in blocks)

    nc = bacc.Bacc(None, target_bir_lowering=False, debug=False)

    xw = nc.dram_tensor("xw", [P, KH * (S * FFS + Ctot)], dt.bfloat16,
                        kind="ExternalInput")
    w2 = nc.dram_tensor("w2", [P, S * KFS * H], dt.bfloat16,
                        kind="ExternalInput")
    b1p = nc.dram_tensor("b1p", [P, S * KFS], dt.float32, kind="ExternalInput")
    y = nc.dram_tensor("y", [P, KH * Ctot], dt.bfloat16, kind="ExternalOutput")

    # xw element offset (per partition) of each segment's packed block;
    # segment 0 occupies [0, KH*(FFS+A0)) split into its pieces.
    seg_off = [0]
    for A in segs:
        seg_off.append(seg_off[-1] + KH * (FFS + A))

    def xw_src(elem_off: int, ncols: int):
        """[p, k, c] view of a contiguous per-partition run of xw."""
        a = elem_off
        return xw[:, a:a + KH * ncols].rearrange("p (k c) -> p k c", k=KH)

    with ExitStack() as ctx:
        tc = ctx.enter_context(tile.TileContext(nc))
        xwpool = ctx.enter_context(tc.tile_pool(name="xwpool", bufs=2))
        w2pool = ctx.enter_context(tc.tile_pool(name="w2pool", bufs=2))
        bpool = ctx.enter_context(tc.tile_pool(name="bpool", bufs=1))
        hpool = ctx.enter_context(tc.tile_pool(name="hpool", bufs=2))
        psA = ctx.enter_context(tc.tile_pool(name="psA", bufs=4, space="PSUM"))
        psB = ctx.enter_context(tc.tile_pool(name="psB", bufs=4, space="PSUM"))
        opool = ctx.enter_context(tc.tile_pool(name="opool", bufs=3))

        act = getattr(mybir.ActivationFunctionType, ACT_OVERRIDE or "Gelu")
        tiles = {}

        b1t = bpool.tile([P, S * KFS], dt.float32, tag="b1t", name="b1t")

        # --- segment 0: one tile per piece, all on the fast sync queue
        # in exact consumption order ---
        blocks0 = blocks_by_seg[0]
        ctW = bpool.tile([P, KH, P], dt.bfloat16, tag="ctW", name="ctW")
        ctB1a = bpool.tile([P, KH, P], dt.bfloat16, tag="ctB1a", name="ctB1a")
        ctB1b = bpool.tile([P, KH, FFS - 2 * P], dt.bfloat16, tag="ctB1b",
                           name="ctB1b")
        xts = [bpool.tile([P, KH, nb], dt.bfloat16, tag=f"xt{bi}",
                          name=f"xt{bi}")
               for bi, (t0, nb) in enumerate(blocks0)]

        def emit_seg0():
            o = 0
            nc.sync.dma_start(out=ctW[:, :, :], in_=xw_src(o, P))
            o += KH * P
            nc.sync.dma_start(out=xts[0][:, :, :], in_=xw_src(o, blocks0[0][1]))
            o += KH * blocks0[0][1]
            nc.gpsimd.dma_start(out=b1t[:], in_=b1p[:, :])
            nc.sync.dma_start(out=ctB1a[:, :, :], in_=xw_src(o, P))
            o += KH * P
            nc.sync.dma_start(out=ctB1b[:, :, :], in_=xw_src(o, FFS - 2 * P))
            o += KH * (FFS - 2 * P)
            for bi in range(1, len(blocks0)):
                nb = blocks0[bi][1]
                nc.sync.dma_start(out=xts[bi][:, :, :], in_=xw_src(o, nb))
                o += KH * nb
            emit_w2(0)

        def emit_w2(si):
            w2t = w2pool.tile([P, KFS, H], dt.bfloat16, tag="w2t",
                              name=f"w2t{si}")
            tiles[("w2", si)] = w2t
            src = w2[:, si * KFS * H:(si + 1) * KFS * H].rearrange(
                "p (k h) -> p k h", k=KFS)
            nc.gpsimd.dma_start(out=w2t[:, :, :], in_=src)

        def emit_seg(si):
            A = segs[si]
            ct = xwpool.tile([P, KH, FFS + A], dt.bfloat16, tag="ct",
                             name=f"ct{si}")
            tiles[("ct", si)] = ct
            nc.sync.dma_start(out=ct[:, :, :], in_=xw_src(seg_off[si], FFS + A))
            emit_w2(si)

        def lhsT1(si, k, ff):
            """GEMM1 stationary operand: w1 ff-chunk (128 cols)."""
            if si == 0:
                if ff == 0:
                    return ctW[:, k, :]
                if ff == 1:
                    return ctB1a[:, k, :]
                return ctB1b[:, k, (ff - 2) * P:(ff - 1) * P]
            ct = tiles[("ct", si)]
            return ct[:, k, ff * P:(ff + 1) * P]

        def rhs1(si, k, bi, t0, nb):
            """GEMM1 moving operand: x token block."""
            if si == 0:
                return xts[bi][:, k, :nb]
            ct = tiles[("ct", si)]
            return ct[:, k, FFS + t0:FFS + t0 + nb]

        # warm-up zero tile first in the vector queue (no input deps) so
        # the PE can start ramping before any DMA lands
        warm = bpool.tile([P, NBMAX], dt.bfloat16, tag="warm", name="warm")
        nc.vector.memset(warm[:, :], 0.0)

        emit_seg0()
        if S > 1:
            emit_seg(1)

        def g1_block(si, hblk, bi, t0, nb):
            """GEMM1 for one token block -> hblk[:, :, t0:t0+nb]."""
            for ff in range(KFS):
                pa = psA.tile([P, NBMAX], dt.float32, tag="pa",
                              name=f"pa{si}_{bi}_{ff}")
                warm_n = 0
                if si == 0 and bi == 0 and ff == 0:
                    # Pre-warm: accumulate zero-matmuls into the first
                    # PSUM group while the first input DMAs land; also
                    # ramps the PE clock out of its cold p-state.
                    warm_n = WARM_N
                    for i in range(warm_n):
                        nc.tensor.matmul(
                            pa[:, :nb],
                            lhsT=warm[:, :P],
                            rhs=warm[:, :nb],
                            start=(i == 0),
                            stop=False,
                        )
                for k in range(KH):
                    nc.tensor.matmul(
                        pa[:, :nb],
                        lhsT=lhsT1(si, k, ff),
                        rhs=rhs1(si, k, bi, t0, nb),
                        start=(k == 0 and warm_n == 0),
                        stop=(k == KH - 1),
                    )
                nc.scalar.activation(
                    hblk[:, ff, t0:t0 + nb],
                    pa[:, :nb],
                    act,
                    bias=b1t[:, si * KFS + ff:si * KFS + ff + 1],
                )

        def g2_block(si, w2t, hblk, bi, t0, nb, last_seg_blocks):
            """GEMM2 for one token block -> packed y DMA."""
            boff = boffs[(si, bi)]
            # ot is PACKED at stride nb so the output DMA is one
            # contiguous [ht][t] run per partition (large descriptors)
            ot = opool.tile([P, KH * NBMAX], dt.bfloat16, tag="ot",
                            name=f"ot{si}_{bi}")
            # tail: the last two blocks drain in staggered row-chunks
            # so only a small transfer trails the final matmul
            nblk_left = last_seg_blocks - bi if si == S - 1 else 99
            if nblk_left == 1:       # final block: 2-ht chunks
                stagger = {1: 0, 3: 2, 5: 4, 7: 6}
            elif nblk_left == 2:     # second-to-last: 4-ht chunks
                stagger = {3: 0, 7: 4}
            else:
                stagger = None
            for ht in range(KH):
                pb = psB.tile([P, NBMAX], dt.float32, tag="pb",
                              name=f"pb{si}_{bi}_{ht}")
                for k in range(KFS):
                    nc.tensor.matmul(
                        pb[:, :nb],
                        lhsT=w2t[:, k, ht * P:(ht + 1) * P],
                        rhs=hblk[:, k, t0:t0 + nb],
                        start=(k == 0),
                        stop=(k == KFS - 1),
                    )
                nc.vector.tensor_copy(ot[:, ht * nb:(ht + 1) * nb],
                                      pb[:, :nb])
                if stagger is not None and ht in stagger:
                    lo = stagger[ht]
                    nc.sync.dma_start(
                        out=y[:, boff + lo * nb:boff + (ht + 1) * nb],
                        in_=ot[:, lo * nb:(ht + 1) * nb],
                    )
            if stagger is None:
                nc.sync.dma_start(
                    out=y[:, boff:boff + KH * nb],
                    in_=ot[:, :KH * nb],
                )

        for si, A in enumerate(segs):
            blocks = blocks_by_seg[si]
            nblk = len(blocks)
            hblk = hpool.tile([P, KFS, A], dt.bfloat16, tag="hblk",
                              name=f"hblk{si}")
            if si == 0:
                # Segment 0 is DMA-arrival-paced: interleave GEMM2 blocks
                # one behind GEMM1 (lag-1) so the PE has ~2x compute per
                # input byte while the head transfers land.
                w2t = tiles.pop(("w2", 0))
                for i in range(nblk + 1):
                    if i < nblk:
                        t0, nb = blocks[i]
                        g1_block(0, hblk, i, t0, nb)
                    if i >= 1:
                        t0, nb = blocks[i - 1]
                        g2_block(0, w2t, hblk, i - 1, t0, nb, nblk)
            else:
                for bi, (t0, nb) in enumerate(blocks):
                    g1_block(si, hblk, bi, t0, nb)
                    if bi == 0 and si + 1 < S:
                        # Prefetch segment si+1 while the rest of this
                        # segment computes (~28us of cover for ~4MB).
                        emit_seg(si + 1)
                w2t = tiles.pop(("w2", si))
                for bi, (t0, nb) in enumerate(blocks):
                    g2_block(si, w2t, hblk, bi, t0, nb, nblk)

    nc.compile()
    return nc


def _get_program(segs: tuple[int, ...]):
    if segs not in _prog_cache:
        _prog_cache[segs] = _build_program(segs)
    return _prog_cache[segs]


def _route(xf: np.ndarray, Wg: np.ndarray, bg: np.ndarray):
    """fp32 gate: softmax -> top-2 (stable order, matches jax top_k) -> renorm."""
    logits = xf @ np.asarray(Wg, np.float32) + np.asarray(bg, np.float32)
    m = logits.max(axis=1, keepdims=True)
    p = np.exp(logits - m, dtype=np.float32)
    p /= p.sum(axis=1, keepdims=True)
    order = np.argsort(-p, axis=1, kind="stable")
    idx = order[:, :TOPK]
    pv = np.take_along_axis(p, idx, axis=1)
    vals = (pv / pv.sum(axis=1, keepdims=True)).astype(np.float32)
    return idx, vals


def _pack_pm(arr_hc: np.ndarray) -> np.ndarray:
    """[H, C] -> partition-major [P, KH*C] (per partition: [k][c])."""
    h, c = arr_hc.shape
    return np.ascontiguousarray(
        arr_hc.reshape(h // P, P, c).transpose(1, 0, 2).reshape(P, -1)
    )


def kernel(x, Wg, bg, W1, b1, W2, b2):
    global LAST_RESULTS, LAST_CALL
    from concourse.bass_utils import run_bass_kernel_spmd

    bf16 = ml_dtypes.bfloat16
    x = np.asarray(x, np.float32)
    xf = x.reshape(-1, H)
    T = xf.shape[0]

    idx, vals = _route(xf, Wg, bg)
    counts = np.bincount(idx.ravel(), minlength=E)

    # Segments: experts by token count (desc), zero-count experts skipped.
    order = [int(e) for e in np.argsort(-counts, kind="stable") if counts[e] > 0]
    segs = tuple(int(counts[e]) for e in order)
    S = len(segs)
    Ctot = sum(segs)
    A0 = segs[0]
    nb0 = min(NB0, A0)
    blocks_by_seg, boffs = _plan(segs)

    nc = _get_program(segs)

    W1 = np.asarray(W1, np.float32)
    W2 = np.asarray(W2, np.float32)
    b1 = np.asarray(b1, np.float32)

    # Token ids / combine scales / packed x^T per segment (shared by cores).
    shards = []
    xparts = []   # per segment: [P, KH*A] partition-major bf16
    for si in range(S):
        e = order[si]
        sel = idx == e                  # [T, 2]; at most one True per row
        ids = np.nonzero(sel.any(axis=1))[0]
        sc = vals[sel]                  # row-major => aligned with ids
        shards.append((ids, sc))
        xparts.append(_pack_pm(xf[ids].T.astype(bf16)))

    in_maps = []
    for c in range(NC):
        pieces = []
        for si in range(S):
            e = order[si]
            w1s = W1[e][:, c * FFS:(c + 1) * FFS].astype(bf16)
            if si == 0:
                # piece order (each k-major, matching its SBUF tile):
                # w1c0 | xblk0 | w1c1 | w1c2.. | xblk1 | xblk2 ..
                x3 = xparts[0].reshape(P, KH, A0)
                pieces.append(_pack_pm(w1s[:, :P]))
                pieces.append(np.ascontiguousarray(x3[:, :, :nb0])
                              .reshape(P, -1))
                pieces.append(_pack_pm(w1s[:, P:2 * P]))
                pieces.append(_pack_pm(w1s[:, 2 * P:]))
                for t0, nb in blocks_by_seg[0][1:]:
                    pieces.append(np.ascontiguousarray(x3[:, :, t0:t0 + nb])
                                  .reshape(P, -1))
            else:
                # per partition: [k][w1 cols | x cols] contiguous
                w13 = _pack_pm(w1s).reshape(P, KH, FFS)
                x3 = xparts[si].reshape(P, KH, segs[si])
                pieces.append(np.concatenate([w13, x3], axis=2)
                              .reshape(P, -1))
        xwc = np.ascontiguousarray(np.concatenate(pieces, axis=1))
        w2c = np.concatenate(
            [_pack_pm(W2[order[si]][c * FFS:(c + 1) * FFS, :].astype(bf16))
             for si in range(S)],
            axis=1,
        )
        b1c = np.ascontiguousarray(np.stack(
            [b1[order[si]][c * FFS + f * P:c * FFS + (f + 1) * P]
             for si in range(S) for f in range(KFS)],
            axis=1,
        ))
        in_maps.append({"xw": xwc, "w2": np.ascontiguousarray(w2c), "b1p": b1c})

    LAST_CALL = (nc, in_maps)
    LAST_RESULTS = run_bass_kernel_spmd(nc, in_maps, list(range(NC)),
                                        trace=TRACE)

    # Sum partials across cores in the packed layout, then unpack.
    ysum2 = np.zeros((P, KH * Ctot), np.float32)
    for c in range(NC):
        ysum2 += LAST_RESULTS.results[c]["y"].astype(np.float32)
    ysum = np.empty((H, Ctot), np.float32)
    soff = 0
    for si in range(S):
        for bi, (t0, nb) in enumerate(blocks_by_seg[si]):
            boff = boffs[(si, bi)]
            blk = ysum2[:, boff:boff + KH * nb].reshape(P, KH, nb)
            ysum[:, soff + t0:soff + t0 + nb] = (
                blk.transpose(1, 0, 2).reshape(H, nb))
        soff += segs[si]

    out = np.zeros((T, H), np.float32)
    c0 = 0
    for si in range(S):
        ids, sc = shards[si]
        out[ids] += ysum[:, c0:c0 + ids.size].T * sc[:, None]
        c0 += segs[si]

    b2 = np.asarray(b2, np.float32)
    out += vals[:, 0:1] * b2[idx[:, 0]] + vals[:, 1:2] * b2[idx[:, 1]]
    return out.reshape(x.shape)
